# revision 13
# baseline (speedup 1.0000x reference)
"""Trainium2 Bass kernel for nn_BiTrap: 8-core data-parallel GRU encoder/decoder.

Layout: channels on SBUF partitions, agents on the free dimension.
All matmuls bf16 (fp32 PSUM accumulation); gate math bf16 on DVE/ACT.
Biases are folded into matmuls via ones-rows riding in the input K-chunks.
"""
import numpy as np
import ml_dtypes

import concourse.bass as bass
import concourse.tile as tile
from concourse import mybir
from concourse.bass_utils import run_bass_kernel_spmd

BF16 = ml_dtypes.bfloat16
F32 = np.float32
DT_BF = mybir.dt.bfloat16
DT_F32 = mybir.dt.float32
AF = mybir.ActivationFunctionType
ALU = mybir.AluOpType

N_CORES = 8
N = 16384
S_OBS = 8
PRE_LEN = 12
GRU = 256
LAT = 32

B = N // N_CORES          # agents per core
Bc = 512                  # agents per PSUM chunk
NB = B // Bc

MAX_WAITS = 1


def _split_excess_waits(nc):
    """walrus rejects >1 sync wait on an instruction; spill extras into
    preceding same-engine NOPs."""
    n_split = 0
    for f in nc.m.functions:
        for blk in f.blocks:
            insts = blk.instructions
            i = 0
            while i < len(insts):
                ins = insts[i]
                si = getattr(ins, "sync_info", None)
                if si is not None and len(si.on_wait) > MAX_WAITS:
                    waits = list(si.on_wait)
                    keep, spill = waits[:MAX_WAITS], waits[MAX_WAITS:]
                    ins.sync_info = mybir.SyncInfo(on_wait=keep, on_update=si.on_update)
                    pos = i
                    while spill:
                        chunk, spill = spill[:MAX_WAITS], spill[MAX_WAITS:]
                        nop = mybir.InstNoOp(
                            name=f"Wsplit-{n_split}",
                            engine=ins.engine,
                            sync_info=mybir.SyncInfo(on_wait=chunk, on_update=[]),
                            bass_nofuse=True,
                        )
                        insts.insert(pos, nop)
                        n_split += 1
                        pos += 1
                        i += 1
                i += 1
    return n_split


# ---------------------------------------------------------------- host prep

def _bf(a):
    return np.ascontiguousarray(np.asarray(a, dtype=np.float32)).astype(BF16)


def _f32(a):
    return np.ascontiguousarray(np.asarray(a, dtype=np.float32))


def _chunk_lhsT(w_t, kc, m):
    # w_t: [K, M] -> [128, kc, m] device layout ([partition, k-chunk, M])
    K, M = w_t.shape
    assert K == kc * 128 and M == m
    return np.ascontiguousarray(w_t.reshape(kc, 128, m).transpose(1, 0, 2))


def _prep_weights(params):
    P = {}

    def gru_pack(tag, g, fc=None):
        wih = _f32(g['wih'])
        whh = _f32(g['whh'])
        bih = _f32(g['bih'])
        bhh = _f32(g['bhh'])
        if fc is not None:
            Wf = wih @ _f32(fc['w'])          # [768, 2]
            cf = wih @ _f32(fc['b'])          # [768]
        else:
            Wf = wih                          # [768, 64]
            cf = np.zeros(768, np.float32)
        brz = (cf + bih + bhh)[:512]
        bin_ = (cf + bih)[512:]
        bhn = bhh[512:]
        P[f'{tag}_wrz_h'] = _bf(_chunk_lhsT(whh[:512].T, 2, 512))
        P[f'{tag}_wrz_i'] = _bf(np.concatenate([Wf[:512].T, brz[None]], 0))
        P[f'{tag}_whn_h'] = _bf(_chunk_lhsT(whh[512:].T, 2, 256))
        P[f'{tag}_whn_b'] = _bf(bhn[None, :])
        P[f'{tag}_win_i'] = _bf(np.concatenate([Wf[512:].T, bin_[None]], 0))

    gru_pack('gx', params['grux'], params['fcx'])
    gru_pack('gy', params['gruy'], params['fcy'])
    gru_pack('gf', params['forward_gru'])
    gru_pack('gb', params['backward_gru'])

    pr = params['prior']
    P['pr_w1'] = _bf(_chunk_lhsT(_f32(pr[0]['w']).T, 2, 128))
    P['pr_b1'] = _f32(pr[0]['b'])[:, None]
    P['pr_w2'] = _bf(_f32(pr[1]['w']).T)
    P['pr_b2'] = _f32(pr[1]['b'])[:, None]
    P['pr_w3'] = _bf(_f32(pr[2]['w']).T)
    P['pr_b3'] = _f32(pr[2]['b'])[:, None]

    re = params['recognition']
    P['re_w1'] = _bf(_chunk_lhsT(_f32(re[0]['w']).T, 4, 256))
    P['re_b1'] = _f32(np.asarray(re[0]['b']).reshape(2, 128).T)  # [128, 2]
    P['re_w2'] = _bf(_chunk_lhsT(_f32(re[1]['w']).T, 2, 128))
    P['re_b2'] = _f32(re[1]['b'])[:, None]
    P['re_w3'] = _bf(_f32(re[2]['w']).T)
    P['re_b3'] = _f32(re[2]['b'])[:, None]

    go = params['goal']
    gw1 = _f32(go[0]['w'])
    P['go_w1h'] = _bf(_chunk_lhsT(gw1[:, :256].T, 2, 128))
    P['go_w1z'] = _bf(np.concatenate([gw1[:, 256:].T, _f32(go[0]['b'])[None]], 0))
    P['go_w2'] = _bf(_f32(go[1]['w']).T)
    P['go_b2'] = _f32(go[1]['b'])[:, None]
    P['go_w3'] = _bf(_f32(go[2]['w']).T)
    P['go_b3'] = _f32(go[2]['b'])[:, None]

    f2 = params['fc2']
    w2 = _f32(f2['w'])
    P['f2_wh'] = _bf(_chunk_lhsT(w2[:, :256].T, 2, 256))
    P['f2_wz'] = _bf(np.concatenate([w2[:, 256:].T, _f32(f2['b'])[None]], 0))

    ff = params['fcf']
    wf = _f32(ff['w'])
    P['ff_wh'] = _bf(_chunk_lhsT(wf[:, :256].T, 2, 64))
    P['ff_wz'] = _bf(np.concatenate([wf[:, 256:].T, _f32(ff['b'])[None]], 0))

    f3 = params['fc3']
    P['f3_w'] = _bf(_chunk_lhsT(_f32(f3['w']).T, 2, 64))
    P['f3_b'] = _f32(f3['b'])[:, None]

    f5 = params['fc5']
    f5g = np.zeros((33, 64), np.float32)
    f5g[0:2] = _f32(f5['w']).T
    f5g[32] = _f32(f5['b'])
    P['f5g_w'] = _bf(f5g)
    P['f5_w'] = _bf(_f32(f5['w']).T)                                             # [2,64]
    P['f5_b'] = _f32(f5['b'])[:, None]

    f6 = params['fc6']
    w6 = _f32(f6['w'])
    P['f6_wf'] = _bf(_chunk_lhsT(w6[:, :256].T, 2, 2))
    P['f6_wb'] = _bf(_chunk_lhsT(w6[:, 256:].T, 2, 2))
    P['f6_b'] = _f32(f6['b'])[:, None]

    P['I2'] = np.eye(2, dtype=np.float32).astype(BF16)
    return P


def _prep_core_inputs(x, y, e, h0x, h0y, c):
    sl = slice(c * B, (c + 1) * B)
    ones = np.ones((1,), np.float32)

    def with_ones(a):  # a: [2, S, B] -> [3, S, B]
        o = np.ones((1,) + a.shape[1:], np.float32)
        return _bf(np.concatenate([a, o], 0))

    def h0_pack(h0):
        ht = _f32(h0[0][sl]).T            # [256, B]
        return _bf(ht.reshape(2, 128, B).transpose(1, 0, 2))  # [128, 2, B]

    return {
        'xin': with_ones(np.asarray(x[0][:, :, sl], np.float32)),
        'yin': with_ones(np.asarray(y[0][:, :, sl], np.float32)),
        'h0x': h0_pack(h0x),
        'h0y': h0_pack(h0y),
        'e_t': _f32(np.asarray(e[sl]).T),
    }


# ---------------------------------------------------------------- device build

WEIGHT_SPECS = None  # filled at build from prepped weights' shapes/dtypes


def _tile(pool, shape, dt, tag, bufs=None):
    return pool.tile(shape, dt, tag=tag, name=tag, bufs=bufs)


def _build_nc(weights):
    nc = bass.Bass()

    wd = {}
    for name, arr in weights.items():
        dt = DT_BF if arr.dtype == BF16 else DT_F32
        wd[name] = nc.declare_dram_parameter(name, list(arr.shape), dt, isOutput=False)

    xin_d = nc.declare_dram_parameter('xin', [3, S_OBS, B], DT_BF, isOutput=False)
    yin_d = nc.declare_dram_parameter('yin', [3, PRE_LEN, B], DT_BF, isOutput=False)
    h0x_d = nc.declare_dram_parameter('h0x', [128, 2, B], DT_BF, isOutput=False)
    h0y_d = nc.declare_dram_parameter('h0y', [128, 2, B], DT_BF, isOutput=False)
    e_d = nc.declare_dram_parameter('e_t', [32, B], DT_F32, isOutput=False)

    outf_d = nc.dram_tensor('outf_d', [PRE_LEN, 2, B], DT_BF)
    p_o = nc.declare_dram_parameter('p_o', [64, B], DT_F32, isOutput=True)
    q_o = nc.declare_dram_parameter('q_o', [64, B], DT_F32, isOutput=True)
    g_o = nc.declare_dram_parameter('g_o', [2, B], DT_F32, isOutput=True)
    bout_o = nc.declare_dram_parameter('bout_o', [PRE_LEN, 2, B], DT_BF, isOutput=True)

    with tile.TileContext(nc) as tc:
        with (
            tc.tile_pool(name="persist", bufs=1) as pp,
            tc.tile_pool(name="xy", bufs=3) as xyp,
            tc.tile_pool(name="gates", bufs=2) as gp,
            tc.tile_pool(name="ps_big", bufs=1, space="PSUM") as ps_big,
            tc.tile_pool(name="ps_med", bufs=2, space="PSUM") as ps_med,
        ):
            # ---- load weights into SBUF (persistent)
            W = {}
            for name, arr in weights.items():
                dt = DT_BF if arr.dtype == BF16 else DT_F32
                t = pp.tile(list(arr.shape), dt, tag=f"w_{name}", name=f"w_{name}")
                nc.sync.dma_start(t[:], wd[name][:])
                W[name] = t

            # ---- persistent activations
            hx = _tile(pp, [128, 2, B], DT_BF, "hx")
            hy = _tile(pp, [128, 2, B], DT_BF, "hy")
            hf = _tile(pp, [128, 2, B], DT_BF, "hf")
            ft = _tile(pp, [65, B], DT_BF, "ft")
            bt = _tile(pp, [65, B], DT_BF, "bt")
            xzc = _tile(pp, [33, B], DT_BF, "xzc")
            gbf = _tile(pp, [33, B], DT_BF, "gbf")
            qm_sb = _tile(pp, [32, B], DT_F32, "qm_sb")
            qs_sb = _tile(pp, [32, B], DT_F32, "qs_sb")
            p_sb = _tile(pp, [64, B], DT_F32, "p_sb")
            g_sb = _tile(pp, [2, B], DT_F32, "g_sb")
            e_sb = _tile(pp, [32, B], DT_F32, "e_sb")
            hnf = _tile(pp, [128, 2, B], DT_BF, "hnf")
            l1r = _tile(pp, [128, 2, B], DT_BF, "l1r")   # recognition L1
            l2s = _tile(pp, [128, B], DT_BF, "l2s")      # shared small MLP tile
            l3s = _tile(pp, [64, B], DT_BF, "l3s")

            nc.sync.dma_start(hx[:], h0x_d[:])
            nc.sync.dma_start(hy[:], h0y_d[:])
            nc.sync.dma_start(e_sb[:], e_d[:])
            nc.vector.memset(ft[64:65, :], 1.0)
            nc.vector.memset(bt[64:65, :], 1.0)
            nc.vector.memset(xzc[32:33, :], 1.0)
            ones_sb = _tile(pp, [1, B], DT_BF, "ones_sb")
            nc.vector.memset(ones_sb[:], 1.0)
            nc.vector.memset(gbf[:], 0.0)
            nc.vector.memset(gbf[32:33, :], 1.0)

            def mm(out, lhsT, rhs, start, stop):
                nc.tensor.matmul(out, lhsT, rhs, start=start, stop=stop)

            # ---------------- GRU step ----------------
            def gru_step(tag, h_cur, in_rhs, blend_h=None, post_chunk=None):
                """in_rhs: AP [Ki, B] bf16, last row ones. blend_h: h used in the
                (h - n) blend (defaults h_cur). post_chunk(j, sl): extra emission
                after h_new chunk j is ready."""
                wrz_h, wrz_i = W[f'{tag}_wrz_h'], W[f'{tag}_wrz_i']
                whn_h, whn_b = W[f'{tag}_whn_h'], W[f'{tag}_whn_b']
                win_i = W[f'{tag}_win_i']
                bh = blend_h if blend_h is not None else h_cur
                for j in range(NB):
                    sl = slice(j * Bc, (j + 1) * Bc)
                    rz_ps = _tile(ps_big, [128, 4, Bc], DT_F32, "rz")
                    for m in range(4):
                        ms = slice(m * 128, (m + 1) * 128)
                        mm(rz_ps[:, m, :], wrz_i[:, ms], in_rhs[:, sl], True, False)
                        mm(rz_ps[:, m, :], wrz_h[:, 0, ms], h_cur[:, 0, sl], False, False)
                        mm(rz_ps[:, m, :], wrz_h[:, 1, ms], h_cur[:, 1, sl], False, True)
                    rz_sb = _tile(gp, [128, 4, Bc], DT_BF, "rzs")
                    nc.scalar.activation(rz_sb[:], rz_ps[:], AF.Sigmoid)

                    hn_ps = _tile(ps_med, [128, 2, Bc], DT_F32, "ps2")
                    for m in range(2):
                        ms = slice(m * 128, (m + 1) * 128)
                        mm(hn_ps[:, m, :], whn_b[:, ms], ones_sb[:, sl], True, False)
                        mm(hn_ps[:, m, :], whn_h[:, 0, ms], h_cur[:, 0, sl], False, False)
                        mm(hn_ps[:, m, :], whn_h[:, 1, ms], h_cur[:, 1, sl], False, True)
                    in_ps = _tile(ps_med, [128, 2, Bc], DT_F32, "ps2")
                    for m in range(2):
                        ms = slice(m * 128, (m + 1) * 128)
                        mm(in_ps[:, m, :], win_i[:, ms], in_rhs[:, sl], True, True)

                    rh = _tile(gp, [128, 2, Bc], DT_BF, "rh")
                    nc.vector.tensor_mul(rh[:], rz_sb[:, 0:2, :], hn_ps[:])
                    t3 = _tile(gp, [128, 2, Bc], DT_BF, "t3")
                    nc.vector.tensor_add(t3[:], in_ps[:], rh[:])
                    n_t = _tile(gp, [128, 2, Bc], DT_BF, "n")
                    nc.scalar.activation(n_t[:], t3[:], AF.Tanh)

                    tmp = _tile(gp, [128, 2, Bc], DT_BF, "tmp")
                    nc.vector.tensor_sub(tmp[:], bh[:, :, sl], n_t[:])
                    tmp2 = _tile(gp, [128, 2, Bc], DT_BF, "tmp2")
                    nc.vector.tensor_mul(tmp2[:], rz_sb[:, 2:4, :], tmp[:])
                    nc.vector.tensor_add(h_cur[:, :, sl], n_t[:], tmp2[:])
                    if post_chunk is not None:
                        post_chunk(j, sl, h_cur)

            # ---------------- encoders (interleaved) ----------------
            for s in range(PRE_LEN):
                if s < S_OBS:
                    xt = _tile(xyp, [3, B], DT_BF, "xt", bufs=2)
                    nc.sync.dma_start(xt[:], xin_d[:, s, :])
                    gru_step('gx', hx, xt)
                yt = _tile(xyp, [3, B], DT_BF, "yt", bufs=2)
                nc.sync.dma_start(yt[:], yin_d[:, s, :])
                gru_step('gy', hy, yt)
            h = hx
            hyf = hy

            # ---------------- prior MLP: p = mlp3(prior, h) ----------------
            ps = _tile(ps_big, [128, 4, Bc], DT_F32, "rz")
            for j in range(NB):
                sl = slice(j * Bc, (j + 1) * Bc)
                mm(ps[:, j, :], W['pr_w1'][:, 0, :], h[:, 0, sl], True, False)
                mm(ps[:, j, :], W['pr_w1'][:, 1, :], h[:, 1, sl], False, True)
            pl1 = _tile(pp, [128, B], DT_BF, "pl1")
            nc.scalar.activation(pl1[:], ps[:], AF.Relu, bias=W['pr_b1'][:])
            ps = _tile(ps_big, [64, 4, Bc], DT_F32, "rz")
            for j in range(NB):
                sl = slice(j * Bc, (j + 1) * Bc)
                mm(ps[:, j, :], W['pr_w2'][:], pl1[:, sl], True, True)
            nc.scalar.activation(l2s[0:64, :], ps[:], AF.Relu, bias=W['pr_b2'][:])
            ps = _tile(ps_big, [64, 4, Bc], DT_F32, "rz")
            for j in range(NB):
                sl = slice(j * Bc, (j + 1) * Bc)
                mm(ps[:, j, :], W['pr_w3'][:], l2s[0:64, sl], True, True)
            nc.scalar.activation(p_sb[:], ps[:], AF.Identity, bias=W['pr_b3'][:])
            nc.sync.dma_start(p_o[:], p_sb[:])

            # ---------------- recognition MLP: q = mlp3(recog, [h; hy]) ----------------
            for m in range(2):
                ms = slice(m * 128, (m + 1) * 128)
                ps = _tile(ps_big, [128, 4, Bc], DT_F32, "rz")
                for j in range(NB):
                    sl = slice(j * Bc, (j + 1) * Bc)
                    mm(ps[:, j, :], W['re_w1'][:, 0, ms], h[:, 0, sl], True, False)
                    mm(ps[:, j, :], W['re_w1'][:, 1, ms], h[:, 1, sl], False, False)
                    mm(ps[:, j, :], W['re_w1'][:, 2, ms], hyf[:, 0, sl], False, False)
                    mm(ps[:, j, :], W['re_w1'][:, 3, ms], hyf[:, 1, sl], False, True)
                nc.scalar.activation(l1r[:, m, :], ps[:], AF.Relu,
                                     bias=W['re_b1'][:, m:m + 1])
            ps = _tile(ps_big, [128, 4, Bc], DT_F32, "rz")
            for j in range(NB):
                sl = slice(j * Bc, (j + 1) * Bc)
                mm(ps[:, j, :], W['re_w2'][:, 0, :], l1r[:, 0, sl], True, False)
                mm(ps[:, j, :], W['re_w2'][:, 1, :], l1r[:, 1, sl], False, True)
            nc.scalar.activation(l2s[:], ps[:], AF.Relu, bias=W['re_b2'][:])
            ps = _tile(ps_big, [32, 4, Bc], DT_F32, "rz")
            for j in range(NB):
                sl = slice(j * Bc, (j + 1) * Bc)
                mm(ps[:, j, :], W['re_w3'][:, 0:32], l2s[:, sl], True, True)
            nc.scalar.activation(qm_sb[:], ps[:], AF.Identity, bias=W['re_b3'][0:32])
            ps = _tile(ps_big, [32, 4, Bc], DT_F32, "rz")
            for j in range(NB):
                sl = slice(j * Bc, (j + 1) * Bc)
                mm(ps[:, j, :], W['re_w3'][:, 32:64], l2s[:, sl], True, True)
            nc.scalar.activation(qs_sb[:], ps[:], AF.Identity, bias=W['re_b3'][32:64])
            nc.sync.dma_start(q_o[0:32, :], qm_sb[:])
            nc.sync.dma_start(q_o[32:64, :], qs_sb[:])

            # ---------------- z = q[:32] + q[32:]*e ; xzc ----------------
            t_e = _tile(gp, [32, B], DT_F32, "te", bufs=1)
            nc.vector.tensor_mul(t_e[:], qs_sb[:], e_sb[:])
            nc.vector.tensor_add(xzc[0:32, :], qm_sb[:], t_e[:])

            # ---------------- goal MLP: g ----------------
            ps = _tile(ps_big, [128, 4, Bc], DT_F32, "rz")
            for j in range(NB):
                sl = slice(j * Bc, (j + 1) * Bc)
                mm(ps[:, j, :], W['go_w1h'][:, 0, :], h[:, 0, sl], True, False)
                mm(ps[:, j, :], W['go_w1h'][:, 1, :], h[:, 1, sl], False, False)
                mm(ps[:, j, :], W['go_w1z'][:], xzc[:, sl], False, True)
            pl1g = _tile(pp, [128, B], DT_BF, "pl1")
            nc.scalar.activation(pl1g[:], ps[:], AF.Relu)
            ps = _tile(ps_big, [64, 4, Bc], DT_F32, "rz")
            for j in range(NB):
                sl = slice(j * Bc, (j + 1) * Bc)
                mm(ps[:, j, :], W['go_w2'][:], pl1g[:, sl], True, True)
            nc.scalar.activation(l3s[:], ps[:], AF.Relu, bias=W['go_b2'][:])
            ps = _tile(ps_big, [2, 4, Bc], DT_F32, "rz")
            for j in range(NB):
                sl = slice(j * Bc, (j + 1) * Bc)
                mm(ps[:, j, :], W['go_w3'][:], l3s[:, sl], True, True)
            nc.scalar.activation(g_sb[:], ps[:], AF.Identity, bias=W['go_b3'][:])
            nc.sync.dma_start(g_o[:], g_sb[:])
            nc.scalar.activation(gbf[0:2, :], ps[:], AF.Identity, bias=W['go_b3'][:])

            # ---------------- fwd_h0 = fc2(xz); f0 = fcf(xz); b0 = fc5(g) ----------------
            for m in range(2):
                ms = slice(m * 128, (m + 1) * 128)
                ps = _tile(ps_big, [128, 4, Bc], DT_F32, "rz")
                for j in range(NB):
                    sl = slice(j * Bc, (j + 1) * Bc)
                    mm(ps[:, j, :], W['f2_wh'][:, 0, ms], h[:, 0, sl], True, False)
                    mm(ps[:, j, :], W['f2_wh'][:, 1, ms], h[:, 1, sl], False, False)
                    mm(ps[:, j, :], W['f2_wz'][:, ms], xzc[:, sl], False, True)
                nc.scalar.activation(hf[:, m, :], ps[:], AF.Identity)
            ps = _tile(ps_big, [64, 4, Bc], DT_F32, "rz")
            for j in range(NB):
                sl = slice(j * Bc, (j + 1) * Bc)
                mm(ps[:, j, :], W['ff_wh'][:, 0, :], h[:, 0, sl], True, False)
                mm(ps[:, j, :], W['ff_wh'][:, 1, :], h[:, 1, sl], False, False)
                mm(ps[:, j, :], W['ff_wz'][:], xzc[:, sl], False, True)
            nc.scalar.activation(ft[0:64, :], ps[:], AF.Identity)
            ps = _tile(ps_big, [64, 4, Bc], DT_F32, "rz")
            for j in range(NB):
                sl = slice(j * Bc, (j + 1) * Bc)
                mm(ps[:, j, :], W['f5g_w'][:], gbf[:, sl], True, True)
            nc.scalar.activation(bt[0:64, :], ps[:], AF.Identity)

            # ---------------- forward scan ----------------
            def fwd_post(t):
                def post(j, sl, h_new):
                    f3ps = _tile(ps_med, [64, Bc], DT_F32, "ps2")
                    mm(f3ps[:], W['f3_w'][:, 0, :], h_new[:, 0, sl], True, False)
                    mm(f3ps[:], W['f3_w'][:, 1, :], h_new[:, 1, sl], False, True)
                    nc.scalar.activation(ft[0:64, sl], f3ps[:],
                                         AF.Identity, bias=W['f3_b'][:])
                    o6ps = _tile(ps_med, [2, Bc], DT_F32, "ps2")
                    mm(o6ps[:], W['f6_wf'][:, 0, :], h_new[:, 0, sl], True, False)
                    mm(o6ps[:], W['f6_wf'][:, 1, :], h_new[:, 1, sl], False, True)
                    of6 = _tile(gp, [2, Bc], DT_BF, "of6", bufs=3)
                    nc.scalar.activation(of6[:], o6ps[:],
                                         AF.Identity, bias=W['f6_b'][:])
                    nc.sync.dma_start(outf_d[t, :, sl], of6[:])
                return post

            for t in range(PRE_LEN):
                gru_step('gf', hf, ft[:], post_chunk=fwd_post(t))
            h_last = hf

            # ---------------- backward precompute ----------------
            for j in range(NB):
                sl = slice(j * Bc, (j + 1) * Bc)
                hn_ps = _tile(ps_med, [128, 2, Bc], DT_F32, "ps2")
                for m in range(2):
                    ms = slice(m * 128, (m + 1) * 128)
                    mm(hn_ps[:, m, :], W['gb_whn_b'][:, ms], ones_sb[:, sl], True, False)
                    mm(hn_ps[:, m, :], W['gb_whn_h'][:, 0, ms], h_last[:, 0, sl], False, False)
                    mm(hn_ps[:, m, :], W['gb_whn_h'][:, 1, ms], h_last[:, 1, sl], False, True)
                nc.scalar.activation(hnf[:, :, sl], hn_ps[:], AF.Identity)

            # ---------------- backward scan ----------------
            for i in range(PRE_LEN):
                t = PRE_LEN - 1 - i
                for j in range(NB):
                    sl = slice(j * Bc, (j + 1) * Bc)
                    rz_ps = _tile(ps_big, [128, 4, Bc], DT_F32, "rz")
                    for m in range(4):
                        ms = slice(m * 128, (m + 1) * 128)
                        mm(rz_ps[:, m, :], W['gb_wrz_i'][:, ms], bt[:, sl], True, False)
                        mm(rz_ps[:, m, :], W['gb_wrz_h'][:, 0, ms], h_last[:, 0, sl], False, False)
                        mm(rz_ps[:, m, :], W['gb_wrz_h'][:, 1, ms], h_last[:, 1, sl], False, True)
                    rz_sb = _tile(gp, [128, 4, Bc], DT_BF, "rzs")
                    nc.scalar.activation(rz_sb[:], rz_ps[:], AF.Sigmoid)
                    in_ps = _tile(ps_med, [128, 2, Bc], DT_F32, "ps2")
                    for m in range(2):
                        ms = slice(m * 128, (m + 1) * 128)
                        mm(in_ps[:, m, :], W['gb_win_i'][:, ms], bt[:, sl], True, True)
                    rh = _tile(gp, [128, 2, Bc], DT_BF, "rh")
                    nc.vector.tensor_mul(rh[:], rz_sb[:, 0:2, :], hnf[:, :, sl])
                    t3 = _tile(gp, [128, 2, Bc], DT_BF, "t3")
                    nc.vector.tensor_add(t3[:], in_ps[:], rh[:])
                    n_t = _tile(gp, [128, 2, Bc], DT_BF, "n")
                    nc.scalar.activation(n_t[:], t3[:], AF.Tanh)
                    tmp = _tile(gp, [128, 2, Bc], DT_BF, "tmp")
                    nc.vector.tensor_sub(tmp[:], h_last[:, :, sl], n_t[:])
                    tmp2 = _tile(gp, [128, 2, Bc], DT_BF, "tmp2")
                    nc.vector.tensor_mul(tmp2[:], rz_sb[:, 2:4, :], tmp[:])
                    bh_t = _tile(gp, [128, 2, Bc], DT_BF, "bh")
                    nc.vector.tensor_add(bh_t[:], n_t[:], tmp2[:])

                    ofin = _tile(gp, [2, Bc], DT_BF, "ofin", bufs=3)
                    nc.sync.dma_start(ofin[:], outf_d[t, :, sl])
                    o_ps = _tile(ps_med, [2, Bc], DT_F32, "ps2")
                    mm(o_ps[:], W['f6_wb'][:, 0, :], bh_t[:, 0, :], True, False)
                    mm(o_ps[:], W['f6_wb'][:, 1, :], bh_t[:, 1, :], False, False)
                    mm(o_ps[:], W['I2'][:], ofin[:], False, True)
                    ob = _tile(gp, [2, Bc], DT_BF, "ob", bufs=3)
                    nc.scalar.activation(ob[:], o_ps[:], AF.Identity)
                    nc.sync.dma_start(bout_o[i, :, sl], ob[:])
                    if i < PRE_LEN - 1:
                        b_ps = _tile(ps_med, [64, Bc], DT_F32, "ps2")
                        mm(b_ps[:], W['f5_w'][:], ob[:], True, True)
                        nc.scalar.activation(bt[0:64, sl], b_ps[:],
                                             AF.Identity, bias=W['f5_b'][:])

    return nc


# ---------------------------------------------------------------- entry point

_CACHE = {}


def kernel(x, y, e, h0x, h0y, params):
    weights = _prep_weights(params)
    if 'nc' not in _CACHE:
        nc = _build_nc(weights)
        _split_excess_waits(nc)
        _CACHE['nc'] = nc
    nc = _CACHE['nc']

    in_maps = []
    for c in range(N_CORES):
        m = dict(weights)
        m.update(_prep_core_inputs(x, y, e, h0x, h0y, c))
        in_maps.append(m)

    import os
    trace = bool(os.environ.get('BASS_TRACE'))
    res = run_bass_kernel_spmd(nc, in_maps, list(range(N_CORES)), trace=trace)
    _CACHE['last_results'] = res

    p = np.concatenate([res.results[c]['p_o'] for c in range(N_CORES)], 1).T
    q = np.concatenate([res.results[c]['q_o'] for c in range(N_CORES)], 1).T
    g = np.concatenate([res.results[c]['g_o'] for c in range(N_CORES)], 1).T
    bout = np.concatenate(
        [np.asarray(res.results[c]['bout_o'], np.float32) for c in range(N_CORES)], 2
    ).transpose(0, 2, 1)[None]
    return (np.ascontiguousarray(p, np.float32),
            np.ascontiguousarray(q, np.float32),
            np.ascontiguousarray(g, np.float32),
            np.ascontiguousarray(bout, np.float32))


# revision 28
# speedup vs baseline: 1136.5508x; 1136.5508x over previous
"""Trainium2 Bass kernel for nn_BiTrap: 8-core data-parallel GRU encoder/decoder.

Layout: channels on SBUF partitions, agents on the free dimension.
All matmuls bf16 (fp32 PSUM accumulation); gate math bf16 on DVE/ACT.
Biases are folded into matmuls via ones-rows riding in the input K-chunks.
"""
import numpy as np
import ml_dtypes

import concourse.bass as bass
import concourse.tile as tile
from concourse import mybir
from concourse.bass_utils import run_bass_kernel_spmd

BF16 = ml_dtypes.bfloat16
F32 = np.float32
DT_BF = mybir.dt.bfloat16
DT_F32 = mybir.dt.float32
AF = mybir.ActivationFunctionType
ALU = mybir.AluOpType

N_CORES = 8
N = 16384
S_OBS = 8
PRE_LEN = 12
GRU = 256
LAT = 32

B = N // N_CORES          # agents per core
Bc = 512                  # agents per PSUM chunk
NB = B // Bc

MAX_WAITS = 1


def _split_excess_waits(nc):
    """walrus rejects >1 sync wait on an instruction; spill extras into
    preceding same-engine NOPs."""
    n_split = 0
    for f in nc.m.functions:
        for blk in f.blocks:
            insts = blk.instructions
            i = 0
            while i < len(insts):
                ins = insts[i]
                si = getattr(ins, "sync_info", None)
                if si is not None and len(si.on_wait) > MAX_WAITS:
                    waits = list(si.on_wait)
                    keep, spill = waits[:MAX_WAITS], waits[MAX_WAITS:]
                    ins.sync_info = mybir.SyncInfo(on_wait=keep, on_update=si.on_update)
                    pos = i
                    while spill:
                        chunk, spill = spill[:MAX_WAITS], spill[MAX_WAITS:]
                        nop = mybir.InstNoOp(
                            name=f"Wsplit-{n_split}",
                            engine=ins.engine,
                            sync_info=mybir.SyncInfo(on_wait=chunk, on_update=[]),
                            bass_nofuse=True,
                        )
                        insts.insert(pos, nop)
                        n_split += 1
                        pos += 1
                        i += 1
                i += 1
    return n_split


# ---------------------------------------------------------------- host prep

def _bf(a):
    return np.ascontiguousarray(np.asarray(a, dtype=np.float32)).astype(BF16)


def _f32(a):
    return np.ascontiguousarray(np.asarray(a, dtype=np.float32))


def _chunk_lhsT(w_t, kc, m):
    # w_t: [K, M] -> [128, kc, m] device layout ([partition, k-chunk, M])
    K, M = w_t.shape
    assert K == kc * 128 and M == m
    return np.ascontiguousarray(w_t.reshape(kc, 128, m).transpose(1, 0, 2))


def _prep_weights(params):
    P = {}

    def gru_pack(tag, g, fc=None):
        wih = _f32(g['wih'])
        whh = _f32(g['whh'])
        bih = _f32(g['bih'])
        bhh = _f32(g['bhh'])
        if fc is not None:
            Wf = wih @ _f32(fc['w'])          # [768, 2]
            cf = wih @ _f32(fc['b'])          # [768]
        else:
            Wf = wih                          # [768, 64]
            cf = np.zeros(768, np.float32)
        brz = (cf + bih + bhh)[:512]
        bin_ = (cf + bih)[512:]
        bhn = bhh[512:]
        P[f'{tag}_wrz_h'] = _bf(_chunk_lhsT(whh[:512].T, 2, 512))
        P[f'{tag}_wrz_i'] = _bf(np.concatenate([Wf[:512].T, brz[None]], 0))
        P[f'{tag}_whn_h'] = _bf(_chunk_lhsT(whh[512:].T, 2, 256))
        if tag == 'gb':
            P[f'{tag}_whn_b'] = _bf(bhn[None, :])
        else:
            P[f'{tag}_bhn'] = _f32(bhn.reshape(2, 128).T)  # [128, 2]
        P[f'{tag}_win_i'] = _bf(np.concatenate([Wf[512:].T, bin_[None]], 0))

    gru_pack('gx', params['grux'], params['fcx'])
    gru_pack('gy', params['gruy'], params['fcy'])
    gru_pack('gf', params['forward_gru'])
    gru_pack('gb', params['backward_gru'])

    pr = params['prior']
    P['pr_w1'] = _bf(_chunk_lhsT(_f32(pr[0]['w']).T, 2, 128))
    P['pr_b1'] = _f32(pr[0]['b'])[:, None]
    P['pr_w2'] = _bf(_f32(pr[1]['w']).T)
    P['pr_b2'] = _f32(pr[1]['b'])[:, None]
    P['pr_w3'] = _bf(_f32(pr[2]['w']).T)
    P['pr_b3'] = _f32(pr[2]['b'])[:, None]

    re = params['recognition']
    P['re_w1'] = _bf(_chunk_lhsT(_f32(re[0]['w']).T, 4, 256))
    P['re_b1'] = _f32(np.asarray(re[0]['b']).reshape(2, 128).T)  # [128, 2]
    P['re_w2'] = _bf(_chunk_lhsT(_f32(re[1]['w']).T, 2, 128))
    P['re_b2'] = _f32(re[1]['b'])[:, None]
    P['re_w3'] = _bf(_f32(re[2]['w']).T)
    P['re_b3'] = _f32(re[2]['b'])[:, None]

    go = params['goal']
    gw1 = _f32(go[0]['w'])
    P['go_w1h'] = _bf(_chunk_lhsT(gw1[:, :256].T, 2, 128))
    P['go_w1z'] = _bf(np.concatenate([gw1[:, 256:].T, _f32(go[0]['b'])[None]], 0))
    P['go_w2'] = _bf(_f32(go[1]['w']).T)
    P['go_b2'] = _f32(go[1]['b'])[:, None]
    P['go_w3'] = _bf(_f32(go[2]['w']).T)
    P['go_b3'] = _f32(go[2]['b'])[:, None]

    f2 = params['fc2']
    w2 = _f32(f2['w'])
    P['f2_wh'] = _bf(_chunk_lhsT(w2[:, :256].T, 2, 256))
    P['f2_wz'] = _bf(np.concatenate([w2[:, 256:].T, _f32(f2['b'])[None]], 0))

    ff = params['fcf']
    wf = _f32(ff['w'])
    P['ff_wh'] = _bf(_chunk_lhsT(wf[:, :256].T, 2, 64))
    P['ff_wz'] = _bf(np.concatenate([wf[:, 256:].T, _f32(ff['b'])[None]], 0))

    f3 = params['fc3']
    P['f3_b'] = _f32(f3['b'])[:, None]

    f5 = params['fc5']
    f5g = np.zeros((33, 64), np.float32)
    f5g[0:2] = _f32(f5['w']).T
    f5g[32] = _f32(f5['b'])
    P['f5g_w'] = _bf(f5g)
    P['f5_w'] = _bf(_f32(f5['w']).T)                                             # [2,64]
    P['f5_b'] = _f32(f5['b'])[:, None]

    f6 = params['fc6']
    w6 = _f32(f6['w'])
    P['f6_wb'] = _bf(_chunk_lhsT(w6[:, 256:].T, 2, 2))
    P['f6_b'] = _f32(f6['b'])[:, None]
    f36 = np.concatenate([_f32(f3['w']).T, w6[:, :256].T], 1)  # [256, 66]
    P['f36_w'] = _bf(_chunk_lhsT(f36, 2, 66))

    P['I2'] = np.eye(2, dtype=np.float32).astype(BF16)
    return P


def _prep_core_inputs(x, y, e, h0x, h0y, c):
    sl = slice(c * B, (c + 1) * B)
    ones = np.ones((1,), np.float32)

    def with_ones(a):  # a: [2, S, B] -> [3, S, B]
        o = np.ones((1,) + a.shape[1:], np.float32)
        return _bf(np.concatenate([a, o], 0))

    def h0_pack(h0):
        ht = _f32(h0[0][sl]).T            # [256, B]
        return _bf(ht.reshape(2, 128, B).transpose(1, 0, 2))  # [128, 2, B]

    return {
        'xin': with_ones(np.asarray(x[0][:, :, sl], np.float32)),
        'yin': with_ones(np.asarray(y[0][:, :, sl], np.float32)),
        'h0x': h0_pack(h0x),
        'h0y': h0_pack(h0y),
        'e_t': _f32(np.asarray(e[sl]).T),
    }


# ---------------------------------------------------------------- device build

def _pack_weights(weights):
    """Group weight arrays by (partition_dim, dtype) into single [P, F] blobs.
    Returns (blobs: dict blobname->np2d, index: name->(blob, off, free_shape))."""
    groups = {}
    index = {}
    for name, arr in weights.items():
        P = arr.shape[0]
        fshape = arr.shape[1:]
        F = int(np.prod(fshape)) if fshape else 1
        key = (P, str(arr.dtype))
        groups.setdefault(key, []).append((name, arr.reshape(P, F), fshape))
    blobs = {}
    for (P, dt), items in groups.items():
        off = 0
        mats = []
        bname = f"wpack_{P}_{'bf' if 'bfloat' in dt else 'f32'}"
        for name, a2, fshape in items:
            index[name] = (bname, off, fshape)
            off += a2.shape[1]
            mats.append(a2)
        blobs[bname] = np.ascontiguousarray(np.concatenate(mats, 1))
    return blobs, index

WEIGHT_SPECS = None  # filled at build from prepped weights' shapes/dtypes


def _tile(pool, shape, dt, tag, bufs=None):
    return pool.tile(shape, dt, tag=tag, name=tag, bufs=bufs)


def _build_nc(weights):
    nc = bass.Bass()

    blobs, windex = _pack_weights(weights)
    wd = {}
    for name, arr in blobs.items():
        dt = DT_BF if arr.dtype == BF16 else DT_F32
        wd[name] = nc.declare_dram_parameter(name, list(arr.shape), dt, isOutput=False)

    xin_d = nc.declare_dram_parameter('xin', [3, S_OBS, B], DT_BF, isOutput=False)
    yin_d = nc.declare_dram_parameter('yin', [3, PRE_LEN, B], DT_BF, isOutput=False)
    h0x_d = nc.declare_dram_parameter('h0x', [128, 2, B], DT_BF, isOutput=False)
    h0y_d = nc.declare_dram_parameter('h0y', [128, 2, B], DT_BF, isOutput=False)
    e_d = nc.declare_dram_parameter('e_t', [32, B], DT_F32, isOutput=False)

    outf_d = nc.dram_tensor('outf_d', [PRE_LEN, 2, B], DT_BF)
    p_o = nc.declare_dram_parameter('p_o', [64, B], DT_F32, isOutput=True)
    q_o = nc.declare_dram_parameter('q_o', [64, B], DT_F32, isOutput=True)
    g_o = nc.declare_dram_parameter('g_o', [2, B], DT_F32, isOutput=True)
    bout_o = nc.declare_dram_parameter('bout_o', [PRE_LEN, 2, B], DT_BF, isOutput=True)

    with tile.TileContext(nc) as tc:
        with (
            tc.tile_pool(name="persist", bufs=1) as pp,
            tc.tile_pool(name="xy", bufs=2) as xyp,
            tc.tile_pool(name="gates", bufs=2) as gp,
            tc.tile_pool(name="psp", bufs=3, space="PSUM") as psp,
            tc.tile_pool(name="psa", bufs=2, space="PSUM") as psa,
        ):
            # ---- load packed weights into SBUF (persistent), view per name
            blob_t = {}
            for name, arr in blobs.items():
                dt = DT_BF if arr.dtype == BF16 else DT_F32
                t = pp.tile(list(arr.shape), dt, tag=f"w_{name}", name=f"w_{name}")
                nc.sync.dma_start(t[:], wd[name][:])
                blob_t[name] = t
            W = {}
            for name, (bname, off, fshape) in windex.items():
                F = int(np.prod(fshape)) if fshape else 1
                v = blob_t[bname][:, off:off + F]
                if len(fshape) == 2:
                    v = v.rearrange("p (a b) -> p a b", a=fshape[0], b=fshape[1])
                W[name] = v

            # ---- persistent activations
            hx = _tile(pp, [128, 2, B], DT_BF, "hx")
            hy = _tile(pp, [128, 2, B], DT_BF, "hy")
            hf = _tile(pp, [128, 2, B], DT_BF, "hf")
            ft = _tile(pp, [65, B], DT_BF, "ft")
            bt = _tile(pp, [65, B], DT_BF, "bt")
            xzc = _tile(pp, [33, B], DT_BF, "xzc")
            gbf = _tile(pp, [33, B], DT_BF, "gbf")
            qm_sb = _tile(pp, [32, B], DT_F32, "qm_sb")
            qs_sb = _tile(pp, [32, B], DT_F32, "qs_sb")
            p_sb = _tile(pp, [64, B], DT_F32, "p_sb")
            g_sb = _tile(pp, [2, B], DT_F32, "g_sb")
            e_sb = _tile(pp, [32, B], DT_F32, "e_sb")
            hnf = _tile(pp, [128, 2, B], DT_BF, "hnf")
            l1r = _tile(pp, [128, 2, B], DT_BF, "l1r")   # recognition L1
            l2s = _tile(pp, [128, B], DT_BF, "l2s")      # shared small MLP tile
            l3s = _tile(pp, [64, B], DT_BF, "l3s")
            pl1 = _tile(pp, [128, B], DT_BF, "pl1")

            nc.sync.dma_start(hx[:], h0x_d[:])
            nc.sync.dma_start(hy[:], h0y_d[:])
            nc.sync.dma_start(e_sb[:], e_d[:])
            nc.vector.memset(ft[64:65, :], 1.0)
            nc.vector.memset(bt[64:65, :], 1.0)
            nc.vector.memset(xzc[32:33, :], 1.0)
            ones_sb = _tile(pp, [1, Bc], DT_BF, "ones_sb")
            nc.vector.memset(ones_sb[:], 1.0)
            nc.vector.memset(gbf[:], 0.0)
            nc.vector.memset(gbf[32:33, :], 1.0)

            def mm(out, lhsT, rhs, start, stop):
                nc.tensor.matmul(out, lhsT, rhs, start=start, stop=stop)

            def ps2():
                return _tile(psp, [128, 2, Bc], DT_F32, "ps2")

            def aux(p=64):
                return _tile(psa, [p, Bc], DT_F32, "aux")

            # ---------------- GRU step ----------------
            def gru_step(tag, h_cur, in_rhs, h_out=None, blend_h=None,
                         post_chunk=None, hn_fixed=None, chunk_out=False):
                """in_rhs: AP [Ki, B] bf16, last row ones. h_cur feeds the
                matmuls; h_out receives the blended state (defaults h_cur,
                in-place). blend_h: h used in the (h - n) blend (defaults
                h_cur). hn_fixed: precomputed (h_n+bhn) bf16 (backward)."""
                if h_out is None and not chunk_out:
                    h_out = h_cur
                wrz_h, wrz_i = W[f'{tag}_wrz_h'], W[f'{tag}_wrz_i']
                win_i = W[f'{tag}_win_i']
                bh = blend_h if blend_h is not None else h_cur
                for j in range(NB):
                    sl = slice(j * Bc, (j + 1) * Bc)
                    r_ps = ps2()
                    for m in range(2):
                        ms = slice(m * 128, (m + 1) * 128)
                        mm(r_ps[:, m, :], wrz_i[:, ms], in_rhs[:, sl], True, False)
                        mm(r_ps[:, m, :], wrz_h[:, 0, ms], h_cur[:, 0, sl], False, False)
                        mm(r_ps[:, m, :], wrz_h[:, 1, ms], h_cur[:, 1, sl], False, True)
                    r_sb = _tile(gp, [128, 2, Bc], DT_BF, "r_sb", bufs=3)
                    nc.scalar.activation(r_sb[:], r_ps[:], AF.Sigmoid)
                    z_ps = ps2()
                    for m in range(2, 4):
                        ms = slice(m * 128, (m + 1) * 128)
                        mm(z_ps[:, m - 2, :], wrz_i[:, ms], in_rhs[:, sl], True, False)
                        mm(z_ps[:, m - 2, :], wrz_h[:, 0, ms], h_cur[:, 0, sl], False, False)
                        mm(z_ps[:, m - 2, :], wrz_h[:, 1, ms], h_cur[:, 1, sl], False, True)
                    if hn_fixed is None:
                        whn_h, bhn = W[f'{tag}_whn_h'], W[f'{tag}_bhn']
                        hn_ps = ps2()
                        for m in range(2):
                            ms = slice(m * 128, (m + 1) * 128)
                            mm(hn_ps[:, m, :], whn_h[:, 0, ms], h_cur[:, 0, sl], True, False)
                            mm(hn_ps[:, m, :], whn_h[:, 1, ms], h_cur[:, 1, sl], False, True)
                    in_ps = ps2()
                    for m in range(2):
                        ms = slice(m * 128, (m + 1) * 128)
                        mm(in_ps[:, m, :], win_i[:, ms], in_rhs[:, sl], True, True)
                    z_sb = _tile(gp, [128, 2, Bc], DT_BF, "z_sb")
                    nc.scalar.activation(z_sb[:], z_ps[:], AF.Sigmoid)

                    rh = _tile(gp, [128, 2, Bc], DT_BF, "rh")
                    if hn_fixed is None:
                        for m in range(2):
                            nc.vector.scalar_tensor_tensor(
                                rh[:, m, :], hn_ps[:, m, :], bhn[:, m:m + 1],
                                r_sb[:, m, :], ALU.add, ALU.mult)
                    else:
                        nc.vector.tensor_mul(rh[:], r_sb[:], hn_fixed[:, :, sl])
                    t3 = _tile(gp, [128, 2, Bc], DT_BF, "t3", bufs=3)
                    nc.vector.tensor_add(t3[:], in_ps[:], rh[:])
                    n_t = _tile(gp, [128, 2, Bc], DT_BF, "n")
                    nc.scalar.activation(n_t[:], t3[:], AF.Tanh)

                    tmp = _tile(gp, [128, 2, Bc], DT_BF, "tmp")
                    nc.vector.tensor_sub(tmp[:], bh[:, :, sl], n_t[:])
                    tmp2 = _tile(gp, [128, 2, Bc], DT_BF, "tmp2")
                    nc.vector.tensor_mul(tmp2[:], z_sb[:], tmp[:])
                    if chunk_out:
                        bh_c = _tile(gp, [128, 2, Bc], DT_BF, "bh_c")
                        nc.vector.tensor_add(bh_c[:], n_t[:], tmp2[:])
                        post_chunk(j, sl, bh_c)
                    else:
                        nc.vector.tensor_add(h_out[:, :, sl], n_t[:], tmp2[:])
                        if post_chunk is not None:
                            post_chunk(j, sl, h_out)

            # ---------------- prior MLP: p = mlp3(prior, h) ----------------
            def emit_prior():
                for jj in range(2):  # pairs of B-chunks
                    s2 = slice(jj * 2 * Bc, (jj + 1) * 2 * Bc)
                    ps = ps2()
                    for j in range(2):
                        sl = slice((2 * jj + j) * Bc, (2 * jj + j + 1) * Bc)
                        mm(ps[:, j, :], W['pr_w1'][:, 0, :], hx[:, 0, sl], True, False)
                        mm(ps[:, j, :], W['pr_w1'][:, 1, :], hx[:, 1, sl], False, True)
                    nc.scalar.activation(pl1[:, s2], ps[:], AF.Relu, bias=W['pr_b1'][:])
                for j in range(NB):
                    sl = slice(j * Bc, (j + 1) * Bc)
                    ps = aux()
                    mm(ps[:], W['pr_w2'][:], pl1[:, sl], True, True)
                    nc.scalar.activation(l2s[0:64, sl], ps[:], AF.Relu, bias=W['pr_b2'][:])
                for j in range(NB):
                    sl = slice(j * Bc, (j + 1) * Bc)
                    ps = aux()
                    mm(ps[:], W['pr_w3'][:], l2s[0:64, sl], True, True)
                    nc.scalar.activation(p_sb[:, sl], ps[:], AF.Identity, bias=W['pr_b3'][:])
                nc.sync.dma_start(p_o[:], p_sb[:])


            # ---------------- encoders (interleaved) ----------------
            for s in range(PRE_LEN):
                if s < S_OBS:
                    xt = _tile(xyp, [3, B], DT_BF, "xt")
                    nc.sync.dma_start(xt[:], xin_d[:, s, :])
                    gru_step('gx', hx, xt)
                yt = _tile(xyp, [3, B], DT_BF, "yt")
                nc.sync.dma_start(yt[:], yin_d[:, s, :])
                gru_step('gy', hy, yt)
                if s == S_OBS:
                    emit_prior()
            h = hx
            hyf = hy

            # ---------------- recognition MLP: q = mlp3(recog, [h; hy]) -------
            for m in range(2):
                ms = slice(m * 128, (m + 1) * 128)
                for jj in range(2):
                    s2 = slice(jj * 2 * Bc, (jj + 1) * 2 * Bc)
                    ps = ps2()
                    for j in range(2):
                        sl = slice((2 * jj + j) * Bc, (2 * jj + j + 1) * Bc)
                        mm(ps[:, j, :], W['re_w1'][:, 0, ms], h[:, 0, sl], True, False)
                        mm(ps[:, j, :], W['re_w1'][:, 1, ms], h[:, 1, sl], False, False)
                        mm(ps[:, j, :], W['re_w1'][:, 2, ms], hyf[:, 0, sl], False, False)
                        mm(ps[:, j, :], W['re_w1'][:, 3, ms], hyf[:, 1, sl], False, True)
                    nc.scalar.activation(l1r[:, m, s2], ps[:], AF.Relu,
                                         bias=W['re_b1'][:, m:m + 1])
            for jj in range(2):
                s2 = slice(jj * 2 * Bc, (jj + 1) * 2 * Bc)
                ps = ps2()
                for j in range(2):
                    sl = slice((2 * jj + j) * Bc, (2 * jj + j + 1) * Bc)
                    mm(ps[:, j, :], W['re_w2'][:, 0, :], l1r[:, 0, sl], True, False)
                    mm(ps[:, j, :], W['re_w2'][:, 1, :], l1r[:, 1, sl], False, True)
                nc.scalar.activation(l2s[:, s2], ps[:], AF.Relu, bias=W['re_b2'][:])
            for j in range(NB):
                sl = slice(j * Bc, (j + 1) * Bc)
                ps = aux(32)
                mm(ps[:], W['re_w3'][:, 0:32], l2s[:, sl], True, True)
                nc.scalar.activation(qm_sb[:, sl], ps[:], AF.Identity,
                                     bias=W['re_b3'][0:32])
                ps = aux(32)
                mm(ps[:], W['re_w3'][:, 32:64], l2s[:, sl], True, True)
                nc.scalar.activation(qs_sb[:, sl], ps[:], AF.Identity,
                                     bias=W['re_b3'][32:64])
            nc.sync.dma_start(q_o[0:32, :], qm_sb[:])
            nc.sync.dma_start(q_o[32:64, :], qs_sb[:])

            # ---------------- z = q[:32] + q[32:]*e ; xzc ----------------
            t_e = _tile(gp, [32, B], DT_BF, "te", bufs=1)
            nc.vector.tensor_mul(t_e[:], qs_sb[:], e_sb[:])
            nc.vector.tensor_add(xzc[0:32, :], qm_sb[:], t_e[:])

            # ---------------- goal MLP: g ----------------
            for jj in range(2):
                s2 = slice(jj * 2 * Bc, (jj + 1) * 2 * Bc)
                ps = ps2()
                for j in range(2):
                    sl = slice((2 * jj + j) * Bc, (2 * jj + j + 1) * Bc)
                    mm(ps[:, j, :], W['go_w1h'][:, 0, :], h[:, 0, sl], True, False)
                    mm(ps[:, j, :], W['go_w1h'][:, 1, :], h[:, 1, sl], False, False)
                    mm(ps[:, j, :], W['go_w1z'][:], xzc[:, sl], False, True)
                nc.scalar.activation(pl1[:, s2], ps[:], AF.Relu)
            for j in range(NB):
                sl = slice(j * Bc, (j + 1) * Bc)
                ps = aux()
                mm(ps[:], W['go_w2'][:], pl1[:, sl], True, True)
                nc.scalar.activation(l3s[:, sl], ps[:], AF.Relu, bias=W['go_b2'][:])
            for j in range(NB):
                sl = slice(j * Bc, (j + 1) * Bc)
                ps = aux(2)
                mm(ps[:], W['go_w3'][:], l3s[:, sl], True, True)
                nc.scalar.activation(g_sb[:, sl], ps[:], AF.Identity, bias=W['go_b3'][:])
                nc.scalar.activation(gbf[0:2, sl], ps[:], AF.Identity, bias=W['go_b3'][:])
            nc.sync.dma_start(g_o[:], g_sb[:])

            # ------------- fwd_h0 = fc2(xz); f0 = fcf(xz); b0 = fc5(g) -------
            for m in range(2):
                ms = slice(m * 128, (m + 1) * 128)
                for jj in range(2):
                    s2 = slice(jj * 2 * Bc, (jj + 1) * 2 * Bc)
                    ps = ps2()
                    for j in range(2):
                        sl = slice((2 * jj + j) * Bc, (2 * jj + j + 1) * Bc)
                        mm(ps[:, j, :], W['f2_wh'][:, 0, ms], h[:, 0, sl], True, False)
                        mm(ps[:, j, :], W['f2_wh'][:, 1, ms], h[:, 1, sl], False, False)
                        mm(ps[:, j, :], W['f2_wz'][:, ms], xzc[:, sl], False, True)
                    nc.scalar.activation(hf[:, m, s2], ps[:], AF.Identity)
            for j in range(NB):
                sl = slice(j * Bc, (j + 1) * Bc)
                ps = aux()
                mm(ps[:], W['ff_wh'][:, 0, :], h[:, 0, sl], True, False)
                mm(ps[:], W['ff_wh'][:, 1, :], h[:, 1, sl], False, False)
                mm(ps[:], W['ff_wz'][:], xzc[:, sl], False, True)
                nc.scalar.activation(ft[0:64, sl], ps[:], AF.Identity)
            for j in range(NB):
                sl = slice(j * Bc, (j + 1) * Bc)
                ps = aux()
                mm(ps[:], W['f5g_w'][:], gbf[:, sl], True, True)
                nc.scalar.activation(bt[0:64, sl], ps[:], AF.Identity)

            # ---------------- forward scan ----------------
            def fwd_post(t):
                cell = {}

                def post(j, sl, h_new):
                    if j == 0:
                        cell['of'] = _tile(gp, [2, B], DT_BF, "of_full", bufs=1)
                    f36ps = aux(66)
                    mm(f36ps[:], W['f36_w'][:, 0, :], h_new[:, 0, sl], True, False)
                    mm(f36ps[:], W['f36_w'][:, 1, :], h_new[:, 1, sl], False, True)
                    nc.scalar.activation(ft[0:64, sl], f36ps[0:64, :],
                                         AF.Identity, bias=W['f3_b'][:])
                    nc.scalar.activation(cell['of'][:, sl], f36ps[64:66, :],
                                         AF.Identity, bias=W['f6_b'][:])
                    if j == NB - 1:
                        nc.sync.dma_start(outf_d[t, :, :], cell['of'][:])
                return post

            for t in range(PRE_LEN):
                gru_step('gf', hf, ft[:], post_chunk=fwd_post(t))
            h_last = hf

            # ---------------- backward precompute hnf ----------------
            for j in range(NB):
                sl = slice(j * Bc, (j + 1) * Bc)
                hn_ps = ps2()
                for m in range(2):
                    ms = slice(m * 128, (m + 1) * 128)
                    mm(hn_ps[:, m, :], W['gb_whn_b'][:, ms], ones_sb[:], True, False)
                    mm(hn_ps[:, m, :], W['gb_whn_h'][:, 0, ms], h_last[:, 0, sl], False, False)
                    mm(hn_ps[:, m, :], W['gb_whn_h'][:, 1, ms], h_last[:, 1, sl], False, True)
                nc.scalar.activation(hnf[:, :, sl], hn_ps[:], AF.Identity)

            # ---------------- backward scan ----------------
            def bwd_post(i, t, ofin, ob_full):
                def post(j, sl, bh_t):
                    o_ps = aux(2)
                    mm(o_ps[:], W['f6_wb'][:, 0, :], bh_t[:, 0, :], True, False)
                    mm(o_ps[:], W['f6_wb'][:, 1, :], bh_t[:, 1, :], False, False)
                    mm(o_ps[:], W['I2'][:], ofin[0:2, sl], False, True)
                    nc.scalar.activation(ob_full[:, sl], o_ps[:], AF.Identity)
                    if j == NB - 1:
                        nc.sync.dma_start(bout_o[i, :, :], ob_full[:])
                    if i < PRE_LEN - 1:
                        b_ps = aux()
                        mm(b_ps[:], W['f5_w'][:], ob_full[0:2, sl], True, True)
                        nc.scalar.activation(bt[0:64, sl], b_ps[:],
                                             AF.Identity, bias=W['f5_b'][:])
                return post

            for i in range(PRE_LEN):
                t = PRE_LEN - 1 - i
                ofin = _tile(gp, [2, B], DT_BF, "ofin_full", bufs=2)
                nc.sync.dma_start(ofin[:], outf_d[t, :, :])
                ob_full = _tile(gp, [2, B], DT_BF, "ob_full", bufs=1)
                gru_step('gb', h_last, bt[:], chunk_out=True, blend_h=h_last,
                         post_chunk=bwd_post(i, t, ofin, ob_full), hn_fixed=hnf)

    return nc


# ---------------------------------------------------------------- entry point

_CACHE = {}


def _get_compiled(weights):
    if 'runner' in _CACHE:
        return _CACHE['runner']
    import jax
    from jax.experimental.shard_map import shard_map
    from jax.sharding import Mesh, PartitionSpec
    from concourse import bass2jax

    nc = _build_nc(weights)
    _split_excess_waits(nc)
    bass2jax.install_neuronx_cc_hook()

    partition_name = nc.partition_id_tensor.name if nc.partition_id_tensor else None
    dbg_name = None
    if nc.dbg_addr is not None:
        assert not nc.dbg_callbacks
        dbg_name = nc.dbg_addr.name
    in_names, out_names, out_avals, zero_outs = [], [], [], []
    for alloc in nc.m.functions[0].allocations:
        if not isinstance(alloc, mybir.MemoryLocationSet):
            continue
        name = alloc.memorylocations[0].name
        if alloc.kind == "ExternalInput":
            if name != partition_name:
                in_names.append(name)
        elif alloc.kind == "ExternalOutput":
            out_names.append(name)
            shape = tuple(alloc.tensor_shape)
            dtype = mybir.dt.np(alloc.dtype)
            out_avals.append(jax.core.ShapedArray(shape, dtype))
            zero_outs.append(np.zeros(shape, dtype))
    n_params = len(in_names)
    all_in = in_names + out_names + ([partition_name] if partition_name else [])

    def _body(*args):
        operands = list(args)
        if partition_name is not None:
            operands.append(bass2jax.partition_id_tensor())
        outs = bass2jax._bass_exec_p.bind(
            *operands,
            out_avals=tuple(out_avals),
            in_names=tuple(all_in),
            out_names=tuple(out_names),
            lowering_input_output_aliases=(),
            sim_require_finite=True,
            sim_require_nnan=True,
            nc=nc,
        )
        return tuple(outs)

    devices = jax.devices()[:N_CORES]
    mesh = Mesh(np.asarray(devices), ("core",))
    in_specs = (PartitionSpec("core"),) * (n_params + len(out_names))
    out_specs = (PartitionSpec("core"),) * len(out_names)
    fn = jax.jit(
        shard_map(_body, mesh=mesh, in_specs=in_specs, out_specs=out_specs,
                  check_rep=False),
        keep_unused=True,
    )
    runner = {
        'fn': fn, 'mesh': mesh, 'in_names': in_names, 'out_names': out_names,
        'out_avals': out_avals, 'zero_outs': zero_outs, 'dbg_name': dbg_name,
    }
    _CACHE['runner'] = runner
    return runner


def _make_args(runner, weights, x, y, e, h0x, h0y):
    blobs, _ = _pack_weights(weights)
    per_core = [None] * N_CORES
    for c in range(N_CORES):
        m = dict(blobs)
        m.update(_prep_core_inputs(x, y, e, h0x, h0y, c))
        if runner['dbg_name'] is not None:
            m[runner['dbg_name']] = np.zeros((1, 2), np.uint32)
        per_core[c] = m
    args = []
    for name in runner['in_names']:
        args.append(np.concatenate([np.asarray(per_core[c][name])
                                    for c in range(N_CORES)], axis=0))
    for z in runner['zero_outs']:
        args.append(np.zeros((N_CORES * z.shape[0], *z.shape[1:]), z.dtype))
    return args


def _run(runner, args):
    import jax
    out_arrs = runner['fn'](*args)
    res = [dict() for _ in range(N_CORES)]
    for i, name in enumerate(runner['out_names']):
        a = np.asarray(out_arrs[i]).reshape(N_CORES, *runner['out_avals'][i].shape)
        for c in range(N_CORES):
            res[c][name] = a[c]
    return res


def kernel(x, y, e, h0x, h0y, params):
    weights = _prep_weights(params)
    runner = _get_compiled(weights)
    args = _make_args(runner, weights, x, y, e, h0x, h0y)
    results = _run(runner, args)

    p = np.concatenate([results[c]['p_o'] for c in range(N_CORES)], 1).T
    q = np.concatenate([results[c]['q_o'] for c in range(N_CORES)], 1).T
    g = np.concatenate([results[c]['g_o'] for c in range(N_CORES)], 1).T
    bout = np.concatenate(
        [np.asarray(results[c]['bout_o'], np.float32) for c in range(N_CORES)], 2
    ).transpose(0, 2, 1)[None]
    return (np.ascontiguousarray(p, np.float32),
            np.ascontiguousarray(q, np.float32),
            np.ascontiguousarray(g, np.float32),
            np.ascontiguousarray(bout, np.float32))
